# revision 27
# baseline (speedup 1.0000x reference)
"""BERT self-attention (B=2, S=2048, H=1024, 16 heads) on 8 TRN2 NeuronCores.

Sharding: tensor-parallel over heads — 2 heads per core. Each core computes
Q/K/V projections for its head slice (contraction over the full hidden dim),
then attention for its (batch, head) pairs, producing the context transposed
[2*64, B*S]. The host concatenates the 8 per-core slices into [B, S, H].

Device-side layout choices (all matmuls fp16 — fp8 was measured to cost
3-6% output error through the softmax's concentrated rows, over the 2%
budget):
  - X is fed pre-transposed ([H, B*S]) so projections run with hidden on the
    partition (contraction) axis; Q^T and K^T come out in [d, token] layout,
    which is exactly what the scores matmul needs.
  - Scores are computed transposed (S^T = K Q^T) per 128-wide k-chunk, two
    heads packed into the PE array concurrently via row tiling (contraction
    is only d=64).
  - exp() runs on the scalar engine straight out of PSUM with the additive
    mask folded into the activation bias and 1/sqrt(d) into its scale.
  - V (+bias) sits in [k, d] fp16 layout with a ones column per head; the
    PV matmul accumulates context and the softmax denominator in one pass.
  - Normalization: native reciprocal on the [1, 512] denominator row (DVE),
    partition-broadcast on the idle GPSIMD engine, one multiply per head,
    fp16 output. No PE broadcast matmul, no PSUM->SBUF staging copy.
  - All DMA issues live on the sync/gpsimd queues so the scalar engine
    (the exp bottleneck) only runs activations.
"""

import sys
import types

sys.path.insert(0, "/opt/trn_rl_repo")

import numpy as np

# NTFF profiling hook (missing from this image's antenv): only needed when
# tracing; install if available, degrade silently otherwise.
try:
    import antenv.axon_hooks  # noqa: F401
except ImportError:
    try:
        from trn_agent_boot.trn_boot import _ntff_profile_via_ctypes

        _m = types.ModuleType("antenv.axon_hooks")
        _hook = _ntff_profile_via_ctypes("/opt/axon/libaxon_pjrt.so")
        _m.get_axon_ntff_profile_hook = lambda: _hook
        _m.set_axon_ntff_profile_hook = lambda h: None
        sys.modules["antenv.axon_hooks"] = _m
    except Exception:
        pass

import concourse.tile as tile
from concourse import bacc, mybir
from concourse.tile_rust import add_dep_helper
from concourse.bass_utils import run_bass_kernel_spmd

F32 = mybir.dt.float32
F16 = mybir.dt.float16
EXP = mybir.ActivationFunctionType.Exp

B, S, H, NHEADS, D = 2, 2048, 1024, 16, 64
T = B * S                # 4096 tokens
DPC = 128                # output dims per core (2 heads x 64)
NCORES = 8
NKC = S // 128           # 16 k-chunks per batch
NQB = S // 512           # 4 q-blocks of 512 per batch
NTB = T // 512           # 8 token blocks of 512
NCI = H // 128           # 8 hidden (contraction) chunks

last_exec_time_ns = None
last_results = None

_cache = {}


def _build():
    nc = bacc.Bacc(
        "TRN2", target_bir_lowering=False, debug=False, enable_asserts=False
    )
    xtt = nc.declare_dram_parameter("xtt", [NTB, 128, NCI, 512], F16,
                                    isOutput=False)
    wq = nc.declare_dram_parameter("wq", [128, NCI, 128], F16, isOutput=False)
    wk = nc.declare_dram_parameter("wk", [128, NCI, 128], F16, isOutput=False)
    wv = nc.declare_dram_parameter("wv", [128, NCI, 128], F16, isOutput=False)
    bq = nc.declare_dram_parameter("bq", [DPC, 1], F32, isOutput=False)
    bk = nc.declare_dram_parameter("bk", [DPC, 1], F32, isOutput=False)
    bvb = nc.declare_dram_parameter("bvb", [128, DPC], F32, isOutput=False)
    msk = nc.declare_dram_parameter("msk", [128, B * NKC], F32, isOutput=False)
    out = nc.declare_dram_parameter("out", [DPC, T], F16, isOutput=True)


    with tile.TileContext(nc) as tc:
        with tc.tile_pool(name="persist", bufs=1) as pp:
            wq_sb = pp.tile([128, NCI, 128], F16, tag="wq")
            wk_sb = pp.tile([128, NCI, 128], F16, tag="wk")
            wv_sb = pp.tile([128, NCI, 128], F16, tag="wv")
            bq_sb = pp.tile([DPC, 1], F32, tag="bq")
            bk_sb = pp.tile([DPC, 1], F32, tag="bk")
            bvb_sb = pp.tile([128, 2, 64], F32, tag="bvb")
            msk_sb = pp.tile([128, B * NKC], F32, tag="msk")
            wu_sb = pp.tile([128, 64], F16, tag="wu")
            qt_sb = pp.tile([128, T], F16, tag="qt")
            kt_sb = pp.tile([128, T], F16, tag="kt")
            # V in [k, d] layout + two ones columns per head (ctx rows
            # 64/65 get the softmax denominator; copied to SBUF base 0 for
            # the custom-DVE reciprocal): [128, b, kc, head, 66] fp16
            vx = pp.tile([128, B, NKC, 2, 66], F16, tag="vx")

            # weights ride the otherwise-idle scalar queue (need order:
            # K first, then Q, then V) while sync+gpsimd stream xt halves
            nc.scalar.dma_start(wk_sb[:], wk[:])
            nc.scalar.dma_start(wq_sb[:], wq[:])
            nc.scalar.dma_start(wv_sb[:], wv[:])
            nc.scalar.dma_start(msk_sb[:], msk[:])
            nc.scalar.dma_start(bq_sb[:], bq[:])
            nc.scalar.dma_start(bk_sb[:], bk[:])
            nc.scalar.dma_start(bvb_sb[:], bvb.rearrange("p (h d) -> p h d", h=2))
            nc.vector.memset(wu_sb[:], 0.0)
            for b_i in range(B):
                for kc in range(NKC):
                    nc.vector.memset(vx[:, b_i, kc, 0, 64:66], 1.0)
                    nc.vector.memset(vx[:, b_i, kc, 1, 64:66], 1.0)

            # ---- Phase 1: projections ----
            # Batch 0 first, K before Q/V, so batch-0 attention (which needs
            # all of K(b0) but only the first q-block of Q) can start early.
            xtp = tc.alloc_tile_pool(name="xtp", bufs=8)

            def dma_xt(tb):
                # every tile split across both queues so delivery order
                # matches the kc consumption order
                xt_t = xtp.tile([128, NCI, 512], F16, tag="xt", name="xt_t")
                nc.sync.dma_start(xt_t[:, 0:4, :], xtt[tb, :, 0:4, :])
                nc.gpsimd.dma_start(xt_t[:, 4:8, :], xtt[tb, :, 4:8, :])
                return xt_t

            b0_tiles = []
            for tb in range(4):
                b0_tiles.append(dma_xt(tb))

            # ---- Phase 2: attention (batch-1 projections woven in) ----
            with tc.tile_pool(name="stp", bufs=2, space="PSUM") as stp, \
                 tc.tile_pool(name="ctxp", bufs=4, space="PSUM") as ctxp, \
                 tc.tile_pool(name="esp", bufs=6) as esp, \
                 tc.tile_pool(name="smallp", bufs=4) as smallp:
                qt_done = {}
                kt_done = {}
                vx_done = {}

                # PE warmup: ~4us of dummy matmuls inside the DMA shadow so
                # the tensor engine reaches its full p-state clock before the
                # first real projection
                warm = ctxp.tile([128, 512], F32, tag="ctx", name="warm")
                for _ in range(200):
                    nc.tensor.matmul(warm[0:64, 0:64], wu_sb[:], wu_sb[:],
                                     start=True, stop=True)

                def proj_qk2(kind, tb, xt_t):
                    w_sb, t_sb, b_sb, done = (
                        (wq_sb, qt_sb, bq_sb, qt_done) if kind == "q"
                        else (wk_sb, kt_sb, bk_sb, kt_done)
                    )
                    ps = ctxp.tile([128, 512], F32, tag="ctx", name="pj_ps0")
                    for ci in range(NCI):
                        nc.tensor.matmul(
                            ps[:], w_sb[:, ci, :], xt_t[:, ci, :],
                            start=(ci == 0), stop=(ci == NCI - 1),
                        )
                    col = tb * 512
                    done[tb] = nc.vector.tensor_scalar_add(
                        t_sb[:, col:col + 512], ps[:], b_sb[:, 0:1]
                    )

                def emit_v_tt(tb, tt):
                    v_ps = ctxp.tile([128, 2, 64], F32, tag="ctx",
                                     name="v_ps")
                    for ci in range(NCI):
                        nc.tensor.matmul(
                            v_ps[:],
                            xt_t_of[tb][:, ci, tt * 128:(tt + 1) * 128],
                            wv_sb[:, ci, :],
                            start=(ci == 0), stop=(ci == NCI - 1),
                        )
                    g = tb * 4 + tt
                    b_i, kc = g // NKC, g % NKC
                    vx_done[(b_i, kc)] = nc.vector.tensor_add(
                        vx[:, b_i, kc, :, 0:64], v_ps[:], bvb_sb[:]
                    )

                def normalize(ctx0, ctx1, qcol):
                    cout = smallp.tile([128, 512], F16, tag="cout",
                                       name="cout")
                    for h, ctx in ((0, ctx0), (1, ctx1)):
                        dsb = smallp.tile([2, 512], F32, tag=f"dsb{h}",
                                          name=f"dsb{h}")
                        nc.vector.tensor_copy(dsb[:], ctx[64:66, :])
                        rb = smallp.tile([2, 512], F32, tag=f"rb{h}",
                                         name=f"rb{h}")
                        nc.vector.reciprocal_approx_fast(rb[:], dsb[:])
                        rbb = smallp.tile([64, 512], F32, tag=f"rbb{h}",
                                          name=f"rbb{h}")
                        nc.gpsimd.partition_broadcast(rbb[:], rb[0:1, :])
                        nc.vector.tensor_mul(
                            cout[h * 64:(h + 1) * 64, :],
                            ctx[0:64, :], rbb[:],
                        )
                    nc.sync.dma_start(out[0:64, qcol:qcol + 512],
                                      cout[0:64, :])
                    nc.gpsimd.dma_start(out[64:128, qcol:qcol + 512],
                                        cout[64:128, :])

                # Work queue of small projection chunks, drained a few
                # matmuls at a time between score chunks so the scalar
                # engine (the bottleneck) is never starved.
                work = []
                xt_t_of = {}

                def wq_proj(kind, tb, xt_t):
                    # split one projection into 4 chunks of 2 ci-steps
                    state = {}

                    def chunk(ci0, kind=kind, tb=tb, xt_t=xt_t, state=state):
                        w_sb, t_sb, b_sb = (
                            (wq_sb, qt_sb, bq_sb) if kind == "q"
                            else (wk_sb, kt_sb, bk_sb)
                        )
                        if ci0 == 0:
                            state["ps"] = ctxp.tile(
                                [128, 512], F32, tag="ctx", name="pj_ps"
                            )
                        ps_t = state["ps"]
                        for ci in (ci0, ci0 + 1):
                            nc.tensor.matmul(
                                ps_t[:], w_sb[:, ci, :], xt_t[:, ci, :],
                                start=(ci == 0), stop=(ci == NCI - 1),
                            )
                        if ci0 == NCI - 2:
                            col = tb * 512
                            done = (qt_done if kind == "q" else kt_done)
                            done[tb] = nc.vector.tensor_scalar_add(
                                t_sb[:, col:col + 512], ps_t[:], b_sb[:, 0:1]
                            )
                    for ci0 in range(0, NCI, 2):
                        work.append(lambda c=ci0: chunk(c))

                def wq_vproj(tb):
                    for tt in range(4):
                        work.append(lambda t=tt, b=tb: emit_v_tt(b, t))

                def filler(b_i, qb):
                    if b_i == 0 and qb == 0:
                        for tt in (2, 3):
                            work.append(lambda t=tt: emit_v_tt(0, t))
                        for tb in (1, 2, 3):
                            wq_proj("k", tb, b0_tiles[tb])
                            wq_vproj(tb)
                        for tb in (1, 2, 3):
                            wq_proj("q", tb, b0_tiles[tb])
                    elif b_i == 0 and qb == 1:
                        for tb in range(4, 8):
                            t = dma_xt(tb)
                            b1_tiles.append(t)
                            xt_t_of[tb] = t
                        for tb in (4, 5, 6, 7):
                            wq_proj("k", tb, b1_tiles[tb - 4])
                    elif b_i == 0 and qb == 2:
                        for tb in (4, 5, 6, 7):
                            wq_vproj(tb)
                        wq_proj("q", 4, b1_tiles[0])
                    elif b_i == 0 and qb == 3:
                        for tb in (5, 6, 7):
                            wq_proj("q", tb, b1_tiles[tb - 4])

                # batch-0 head-start projections
                for tb in range(4):
                    xt_t_of[tb] = b0_tiles[tb]
                proj_qk2("k", 0, b0_tiles[0])
                proj_qk2("q", 0, b0_tiles[0])
                emit_v_tt(0, 0)
                emit_v_tt(0, 1)

                b1_tiles = []

                def emit_scores(b_i, qb, kc, est_of):
                    # dependency drain: make sure kt/qt/vx writers exist
                    ktb = b_i * 4 + kc // 4
                    qtb = b_i * 4 + qb
                    while work and not (
                        ktb in kt_done and qtb in qt_done
                        and (b_i, kc) in vx_done
                    ):
                        work.pop(0)()
                    qcol = b_i * S + qb * 512
                    kcol = b_i * S + kc * 128
                    st = stp.tile([128, 1024], F32, tag="st")
                    m0 = nc.tensor.matmul(
                        st[:, 0:512],
                        kt_sb[0:64, kcol:kcol + 128],
                        qt_sb[0:64, qcol:qcol + 512],
                        start=True, stop=True, tile_position=(0, 0),
                    )
                    m1 = nc.tensor.matmul(
                        st[:, 512:1024],
                        kt_sb[64:128, kcol:kcol + 128],
                        qt_sb[64:128, qcol:qcol + 512],
                        start=True, stop=True, tile_position=(64, 0),
                    )
                    for m in (m0, m1):
                        add_dep_helper(m.ins, kt_done[ktb].ins,
                                       True, "kt ready")
                        add_dep_helper(m.ins, qt_done[qtb].ins,
                                       True, "qt ready")
                    est = esp.tile([128, 1024], F16, tag="est", name="est")
                    est_of[kc] = est
                    nc.scalar.activation(
                        est[:], st[:], EXP, scale=0.125,
                        bias=msk_sb[:, b_i * NKC + kc: b_i * NKC + kc + 1],
                    )

                def emit_pv(b_i, kc, ctx0, ctx1, est_of):
                    est = est_of.pop(kc)
                    p0 = nc.tensor.matmul(
                        ctx0[:], vx[:, b_i, kc, 0, :], est[:, 0:512],
                        start=(kc == 0), stop=(kc == NKC - 1),
                    )
                    p1 = nc.tensor.matmul(
                        ctx1[:], vx[:, b_i, kc, 1, :], est[:, 512:1024],
                        start=(kc == 0), stop=(kc == NKC - 1),
                    )
                    vd = vx_done[(b_i, kc)]
                    add_dep_helper(p0.ins, vd.ins, True, "vx")
                    add_dep_helper(p1.ins, vd.ins, True, "vx")

                # Software-pipelined over q-blocks via a deferred-PV fifo:
                # each scores+exp issue pops at most one trailing PV, so PV
                # (and the normalize chain it gates through the ctx slots)
                # lags scores by PVD k-chunks even across block boundaries —
                # the scalar engine keeps an est backlog and never starves.
                PVD = 4
                fifo = []

                def pop_pv():
                    b_p, kc_p, c0, c1, q_p, eo = fifo.pop(0)
                    emit_pv(b_p, kc_p, c0, c1, eo)
                    if kc_p == NKC - 1:
                        normalize(c0, c1, q_p)

                for b_i in range(B):
                    for qb in range(NQB):
                        filler(b_i, qb)
                        qcol = b_i * S + qb * 512
                        ctx0 = ctxp.tile([66, 512], F32, tag="ctx")
                        ctx1 = ctxp.tile([66, 512], F32, tag="ctx")
                        est_of = {}
                        for kc in range(NKC):
                            emit_scores(b_i, qb, kc, est_of)
                            fifo.append((b_i, kc, ctx0, ctx1, qcol, est_of))
                            if len(fifo) > PVD:
                                pop_pv()
                            for _ in range(2):
                                if work:
                                    work.pop(0)()
                while fifo:
                    pop_pv()
                while work:
                    work.pop(0)()
            xtp.release()

    nc.compile()
    return nc


def kernel(hidden_states, attention_mask, Wq, bq, Wk, bk, Wv, bv, trace=False):
    global last_exec_time_ns, last_results
    x = np.asarray(hidden_states, dtype=np.float32)
    mask = np.asarray(attention_mask, dtype=np.float32)
    Wq = np.asarray(Wq, dtype=np.float32)
    Wk = np.asarray(Wk, dtype=np.float32)
    Wv = np.asarray(Wv, dtype=np.float32)
    bq = np.asarray(bq, dtype=np.float32)
    bk = np.asarray(bk, dtype=np.float32)
    bv = np.asarray(bv, dtype=np.float32)

    if "nc" not in _cache:
        _cache["nc"] = _build()
    nc = _cache["nc"]

    # xt tiles pre-arranged to the SBUF layout: [tb, p, ci, 512]
    xt = x.reshape(T, H).T.astype(np.float16)                        # [H, T]
    xtt = np.ascontiguousarray(
        xt.reshape(NCI, 128, NTB, 512).transpose(2, 1, 0, 3)
    )
    # mask columns: [p, b*16+kc] = mask[b, kc*128+p]
    mcols = np.ascontiguousarray(
        mask.reshape(B, NKC, 128).transpose(2, 0, 1).reshape(128, B * NKC)
    )
    in_maps = []
    for c in range(NCORES):
        sl = slice(c * DPC, (c + 1) * DPC)
        def warr(W):
            return np.ascontiguousarray(
                W[:, sl].astype(np.float16).reshape(NCI, 128, DPC)
                .transpose(1, 0, 2)
            )
        in_maps.append({
            "xtt": xtt,
            "wq": warr(Wq),
            "wk": warr(Wk),
            "wv": warr(Wv),
            "bq": np.ascontiguousarray(bq[sl, None]),
            "bk": np.ascontiguousarray(bk[sl, None]),
            "bvb": np.ascontiguousarray(
                np.broadcast_to(bv[sl][None, :], (128, DPC))
            ),
            "msk": mcols,
        })

    res = run_bass_kernel_spmd(
        nc, in_maps, core_ids=list(range(NCORES)), trace=trace
    )
    last_exec_time_ns = res.exec_time_ns
    last_results = res

    # assemble: per-core out [128, T] -> [B, S, 128]; concat over cores
    parts = [
        res.results[c]["out"].astype(np.float32).reshape(DPC, B, S)
        .transpose(1, 2, 0)
        for c in range(NCORES)
    ]
    return np.ascontiguousarray(np.concatenate(parts, axis=2))


# revision 28
# speedup vs baseline: 1.0166x; 1.0166x over previous
"""BERT self-attention (B=2, S=2048, H=1024, 16 heads) on 8 TRN2 NeuronCores.

Sharding: tensor-parallel over heads — 2 heads per core. Each core computes
Q/K/V projections for its head slice (contraction over the full hidden dim),
then attention for its (batch, head) pairs, producing the context transposed
[2*64, B*S]. The host concatenates the 8 per-core slices into [B, S, H].

Device-side layout choices (all matmuls fp16 — fp8 was measured to cost
3-6% output error through the softmax's concentrated rows, over the 2%
budget):
  - X is fed pre-transposed ([H, B*S]) so projections run with hidden on the
    partition (contraction) axis; Q^T and K^T come out in [d, token] layout,
    which is exactly what the scores matmul needs.
  - Scores are computed transposed (S^T = K Q^T) per 128-wide k-chunk, two
    heads packed into the PE array concurrently via row tiling (contraction
    is only d=64).
  - exp() runs on the scalar engine straight out of PSUM with the additive
    mask folded into the activation bias and 1/sqrt(d) into its scale.
  - V (+bias) sits in [k, d] fp16 layout with a ones column per head; the
    PV matmul accumulates context and the softmax denominator in one pass.
  - Normalization: native reciprocal on the [1, 512] denominator row (DVE),
    partition-broadcast on the idle GPSIMD engine, one multiply per head,
    fp16 output. No PE broadcast matmul, no PSUM->SBUF staging copy.
  - All DMA issues live on the sync/gpsimd queues so the scalar engine
    (the exp bottleneck) only runs activations.
"""

import sys
import types

sys.path.insert(0, "/opt/trn_rl_repo")

import numpy as np

# NTFF profiling hook (missing from this image's antenv): only needed when
# tracing; install if available, degrade silently otherwise.
try:
    import antenv.axon_hooks  # noqa: F401
except ImportError:
    try:
        from trn_agent_boot.trn_boot import _ntff_profile_via_ctypes

        _m = types.ModuleType("antenv.axon_hooks")
        _hook = _ntff_profile_via_ctypes("/opt/axon/libaxon_pjrt.so")
        _m.get_axon_ntff_profile_hook = lambda: _hook
        _m.set_axon_ntff_profile_hook = lambda h: None
        sys.modules["antenv.axon_hooks"] = _m
    except Exception:
        pass

import concourse.tile as tile
from concourse import bacc, mybir
from concourse.tile_rust import add_dep_helper
from concourse.bass_utils import run_bass_kernel_spmd

F32 = mybir.dt.float32
F16 = mybir.dt.float16
EXP = mybir.ActivationFunctionType.Exp

B, S, H, NHEADS, D = 2, 2048, 1024, 16, 64
T = B * S                # 4096 tokens
DPC = 128                # output dims per core (2 heads x 64)
NCORES = 8
NKC = S // 128           # 16 k-chunks per batch
NQB = S // 512           # 4 q-blocks of 512 per batch
NTB = T // 512           # 8 token blocks of 512
NCI = H // 128           # 8 hidden (contraction) chunks

last_exec_time_ns = None
last_results = None

_cache = {}


def _build():
    nc = bacc.Bacc(
        "TRN2", target_bir_lowering=False, debug=False, enable_asserts=False
    )
    xtt = nc.declare_dram_parameter("xtt", [NTB, 128, NCI, 512], F16,
                                    isOutput=False)
    wq = nc.declare_dram_parameter("wq", [128, NCI, 128], F16, isOutput=False)
    wk = nc.declare_dram_parameter("wk", [128, NCI, 128], F16, isOutput=False)
    wv = nc.declare_dram_parameter("wv", [128, NCI, 128], F16, isOutput=False)
    bq = nc.declare_dram_parameter("bq", [DPC, 1], F32, isOutput=False)
    bk = nc.declare_dram_parameter("bk", [DPC, 1], F32, isOutput=False)
    bvb = nc.declare_dram_parameter("bvb", [128, DPC], F32, isOutput=False)
    msk = nc.declare_dram_parameter("msk", [128, B * NKC], F32, isOutput=False)
    out = nc.declare_dram_parameter("out", [DPC, T], F16, isOutput=True)


    with tile.TileContext(nc) as tc:
        with tc.tile_pool(name="persist", bufs=1) as pp:
            wq_sb = pp.tile([128, NCI, 128], F16, tag="wq")
            wk_sb = pp.tile([128, NCI, 128], F16, tag="wk")
            wv_sb = pp.tile([128, NCI, 128], F16, tag="wv")
            bq_sb = pp.tile([DPC, 1], F32, tag="bq")
            bk_sb = pp.tile([DPC, 1], F32, tag="bk")
            bvb_sb = pp.tile([128, 2, 64], F32, tag="bvb")
            msk_sb = pp.tile([128, B * NKC], F32, tag="msk")
            wu_sb = pp.tile([128, 64], F16, tag="wu")
            qt_sb = pp.tile([128, T], F16, tag="qt")
            kt_sb = pp.tile([128, T], F16, tag="kt")
            # V in [k, d] layout + two ones columns per head (ctx rows
            # 64/65 get the softmax denominator; copied to SBUF base 0 for
            # the custom-DVE reciprocal): [128, b, kc, head, 66] fp16
            vx = pp.tile([128, B, NKC, 2, 66], F16, tag="vx")

            # weights ride the otherwise-idle scalar queue (need order:
            # K first, then Q, then V) while sync+gpsimd stream xt halves
            nc.scalar.dma_start(wk_sb[:], wk[:])
            nc.scalar.dma_start(wq_sb[:], wq[:])
            nc.scalar.dma_start(wv_sb[:], wv[:])
            nc.scalar.dma_start(msk_sb[:], msk[:])
            nc.scalar.dma_start(bq_sb[:], bq[:])
            nc.scalar.dma_start(bk_sb[:], bk[:])
            nc.scalar.dma_start(bvb_sb[:], bvb.rearrange("p (h d) -> p h d", h=2))
            nc.vector.memset(wu_sb[:], 0.0)
            for b_i in range(B):
                for kc in range(NKC):
                    nc.vector.memset(vx[:, b_i, kc, 0, 64:66], 1.0)
                    nc.vector.memset(vx[:, b_i, kc, 1, 64:66], 1.0)

            # ---- Phase 1: projections ----
            # Batch 0 first, K before Q/V, so batch-0 attention (which needs
            # all of K(b0) but only the first q-block of Q) can start early.
            xtp = tc.alloc_tile_pool(name="xtp", bufs=8)

            def dma_xt(tb):
                # every tile split across both queues so delivery order
                # matches the kc consumption order
                xt_t = xtp.tile([128, NCI, 512], F16, tag="xt", name="xt_t")
                nc.sync.dma_start(xt_t[:, 0:4, :], xtt[tb, :, 0:4, :])
                nc.gpsimd.dma_start(xt_t[:, 4:8, :], xtt[tb, :, 4:8, :])
                return xt_t

            b0_tiles = []
            for tb in range(4):
                b0_tiles.append(dma_xt(tb))

            # ---- Phase 2: attention (batch-1 projections woven in) ----
            with tc.tile_pool(name="stp", bufs=2, space="PSUM") as stp, \
                 tc.tile_pool(name="ctxp", bufs=4, space="PSUM") as ctxp, \
                 tc.tile_pool(name="esp", bufs=6) as esp, \
                 tc.tile_pool(name="smallp", bufs=4) as smallp:
                qt_done = {}
                kt_done = {}
                vx_done = {}

                # PE warmup: ~4us of dummy matmuls inside the DMA shadow so
                # the tensor engine reaches its full p-state clock before the
                # first real projection
                warm = ctxp.tile([128, 512], F32, tag="ctx", name="warm")
                for _ in range(100):
                    nc.tensor.matmul(warm[0:64, 0:64], wu_sb[:], wu_sb[:],
                                     start=True, stop=True)

                def proj_qk2(kind, tb, xt_t):
                    w_sb, t_sb, b_sb, done = (
                        (wq_sb, qt_sb, bq_sb, qt_done) if kind == "q"
                        else (wk_sb, kt_sb, bk_sb, kt_done)
                    )
                    ps = ctxp.tile([128, 512], F32, tag="ctx", name="pj_ps0")
                    for ci in range(NCI):
                        nc.tensor.matmul(
                            ps[:], w_sb[:, ci, :], xt_t[:, ci, :],
                            start=(ci == 0), stop=(ci == NCI - 1),
                        )
                    col = tb * 512
                    done[tb] = nc.vector.tensor_scalar_add(
                        t_sb[:, col:col + 512], ps[:], b_sb[:, 0:1]
                    )

                def emit_v_tt(tb, tt):
                    v_ps = ctxp.tile([128, 2, 64], F32, tag="ctx",
                                     name="v_ps")
                    for ci in range(NCI):
                        nc.tensor.matmul(
                            v_ps[:],
                            xt_t_of[tb][:, ci, tt * 128:(tt + 1) * 128],
                            wv_sb[:, ci, :],
                            start=(ci == 0), stop=(ci == NCI - 1),
                        )
                    g = tb * 4 + tt
                    b_i, kc = g // NKC, g % NKC
                    vx_done[(b_i, kc)] = nc.vector.tensor_add(
                        vx[:, b_i, kc, :, 0:64], v_ps[:], bvb_sb[:]
                    )

                def normalize(ctx0, ctx1, qcol):
                    cout = smallp.tile([128, 512], F16, tag="cout",
                                       name="cout")
                    for h, ctx in ((0, ctx0), (1, ctx1)):
                        dsb = smallp.tile([2, 512], F32, tag=f"dsb{h}",
                                          name=f"dsb{h}")
                        nc.vector.tensor_copy(dsb[:], ctx[64:66, :])
                        rb = smallp.tile([2, 512], F32, tag=f"rb{h}",
                                         name=f"rb{h}")
                        nc.vector.reciprocal_approx_fast(rb[:], dsb[:])
                        rbb = smallp.tile([64, 512], F32, tag=f"rbb{h}",
                                          name=f"rbb{h}")
                        nc.gpsimd.partition_broadcast(rbb[:], rb[0:1, :])
                        nc.vector.tensor_mul(
                            cout[h * 64:(h + 1) * 64, :],
                            ctx[0:64, :], rbb[:],
                        )
                    nc.sync.dma_start(out[0:64, qcol:qcol + 512],
                                      cout[0:64, :])
                    nc.gpsimd.dma_start(out[64:128, qcol:qcol + 512],
                                        cout[64:128, :])

                # Work queue of small projection chunks, drained a few
                # matmuls at a time between score chunks so the scalar
                # engine (the bottleneck) is never starved.
                work = []
                xt_t_of = {}

                def wq_proj(kind, tb, xt_t):
                    # split one projection into 4 chunks of 2 ci-steps
                    state = {}

                    def chunk(ci0, kind=kind, tb=tb, xt_t=xt_t, state=state):
                        w_sb, t_sb, b_sb = (
                            (wq_sb, qt_sb, bq_sb) if kind == "q"
                            else (wk_sb, kt_sb, bk_sb)
                        )
                        if ci0 == 0:
                            state["ps"] = ctxp.tile(
                                [128, 512], F32, tag="ctx", name="pj_ps"
                            )
                        ps_t = state["ps"]
                        for ci in (ci0, ci0 + 1):
                            nc.tensor.matmul(
                                ps_t[:], w_sb[:, ci, :], xt_t[:, ci, :],
                                start=(ci == 0), stop=(ci == NCI - 1),
                            )
                        if ci0 == NCI - 2:
                            col = tb * 512
                            done = (qt_done if kind == "q" else kt_done)
                            done[tb] = nc.vector.tensor_scalar_add(
                                t_sb[:, col:col + 512], ps_t[:], b_sb[:, 0:1]
                            )
                    for ci0 in range(0, NCI, 2):
                        work.append(lambda c=ci0: chunk(c))

                def wq_vproj(tb):
                    for tt in range(4):
                        work.append(lambda t=tt, b=tb: emit_v_tt(b, t))

                def filler(b_i, qb):
                    if b_i == 0 and qb == 0:
                        for tt in (2, 3):
                            work.append(lambda t=tt: emit_v_tt(0, t))
                        for tb in (1, 2, 3):
                            wq_proj("k", tb, b0_tiles[tb])
                            wq_vproj(tb)
                        for tb in (1, 2, 3):
                            wq_proj("q", tb, b0_tiles[tb])
                    elif b_i == 0 and qb == 1:
                        for tb in range(4, 8):
                            t = dma_xt(tb)
                            b1_tiles.append(t)
                            xt_t_of[tb] = t
                        for tb in (4, 5, 6, 7):
                            wq_proj("k", tb, b1_tiles[tb - 4])
                    elif b_i == 0 and qb == 2:
                        for tb in (4, 5, 6, 7):
                            wq_vproj(tb)
                        wq_proj("q", 4, b1_tiles[0])
                    elif b_i == 0 and qb == 3:
                        for tb in (5, 6, 7):
                            wq_proj("q", tb, b1_tiles[tb - 4])

                # batch-0 head-start projections
                for tb in range(4):
                    xt_t_of[tb] = b0_tiles[tb]
                proj_qk2("k", 0, b0_tiles[0])
                proj_qk2("q", 0, b0_tiles[0])
                emit_v_tt(0, 0)
                emit_v_tt(0, 1)

                b1_tiles = []

                def emit_scores(b_i, qb, kc, est_of):
                    # dependency drain: make sure kt/qt/vx writers exist
                    ktb = b_i * 4 + kc // 4
                    qtb = b_i * 4 + qb
                    while work and not (
                        ktb in kt_done and qtb in qt_done
                        and (b_i, kc) in vx_done
                    ):
                        work.pop(0)()
                    qcol = b_i * S + qb * 512
                    kcol = b_i * S + kc * 128
                    st = stp.tile([128, 1024], F32, tag="st")
                    m0 = nc.tensor.matmul(
                        st[:, 0:512],
                        kt_sb[0:64, kcol:kcol + 128],
                        qt_sb[0:64, qcol:qcol + 512],
                        start=True, stop=True, tile_position=(0, 0),
                    )
                    m1 = nc.tensor.matmul(
                        st[:, 512:1024],
                        kt_sb[64:128, kcol:kcol + 128],
                        qt_sb[64:128, qcol:qcol + 512],
                        start=True, stop=True, tile_position=(64, 0),
                    )
                    for m in (m0, m1):
                        add_dep_helper(m.ins, kt_done[ktb].ins,
                                       True, "kt ready")
                        add_dep_helper(m.ins, qt_done[qtb].ins,
                                       True, "qt ready")
                    est = esp.tile([128, 1024], F16, tag="est", name="est")
                    est_of[kc] = est
                    nc.scalar.activation(
                        est[:], st[:], EXP, scale=0.125,
                        bias=msk_sb[:, b_i * NKC + kc: b_i * NKC + kc + 1],
                    )

                def emit_pv(b_i, kc, ctx0, ctx1, est_of):
                    est = est_of.pop(kc)
                    p0 = nc.tensor.matmul(
                        ctx0[:], vx[:, b_i, kc, 0, :], est[:, 0:512],
                        start=(kc == 0), stop=(kc == NKC - 1),
                    )
                    p1 = nc.tensor.matmul(
                        ctx1[:], vx[:, b_i, kc, 1, :], est[:, 512:1024],
                        start=(kc == 0), stop=(kc == NKC - 1),
                    )
                    vd = vx_done[(b_i, kc)]
                    add_dep_helper(p0.ins, vd.ins, True, "vx")
                    add_dep_helper(p1.ins, vd.ins, True, "vx")

                # Software-pipelined over q-blocks via a deferred-PV fifo:
                # each scores+exp issue pops at most one trailing PV, so PV
                # (and the normalize chain it gates through the ctx slots)
                # lags scores by PVD k-chunks even across block boundaries —
                # the scalar engine keeps an est backlog and never starves.
                PVD = 4
                fifo = []

                def pop_pv():
                    b_p, kc_p, c0, c1, q_p, eo = fifo.pop(0)
                    emit_pv(b_p, kc_p, c0, c1, eo)
                    if kc_p == NKC - 1:
                        normalize(c0, c1, q_p)

                for b_i in range(B):
                    for qb in range(NQB):
                        filler(b_i, qb)
                        qcol = b_i * S + qb * 512
                        ctx0 = ctxp.tile([66, 512], F32, tag="ctx")
                        ctx1 = ctxp.tile([66, 512], F32, tag="ctx")
                        est_of = {}
                        for kc in range(NKC):
                            emit_scores(b_i, qb, kc, est_of)
                            fifo.append((b_i, kc, ctx0, ctx1, qcol, est_of))
                            if len(fifo) > PVD:
                                pop_pv()
                            for _ in range(2):
                                if work:
                                    work.pop(0)()
                while fifo:
                    pop_pv()
                while work:
                    work.pop(0)()
            xtp.release()

    nc.compile()
    return nc


def kernel(hidden_states, attention_mask, Wq, bq, Wk, bk, Wv, bv, trace=False):
    global last_exec_time_ns, last_results
    x = np.asarray(hidden_states, dtype=np.float32)
    mask = np.asarray(attention_mask, dtype=np.float32)
    Wq = np.asarray(Wq, dtype=np.float32)
    Wk = np.asarray(Wk, dtype=np.float32)
    Wv = np.asarray(Wv, dtype=np.float32)
    bq = np.asarray(bq, dtype=np.float32)
    bk = np.asarray(bk, dtype=np.float32)
    bv = np.asarray(bv, dtype=np.float32)

    if "nc" not in _cache:
        _cache["nc"] = _build()
    nc = _cache["nc"]

    # xt tiles pre-arranged to the SBUF layout: [tb, p, ci, 512]
    xt = x.reshape(T, H).T.astype(np.float16)                        # [H, T]
    xtt = np.ascontiguousarray(
        xt.reshape(NCI, 128, NTB, 512).transpose(2, 1, 0, 3)
    )
    # mask columns: [p, b*16+kc] = mask[b, kc*128+p]
    mcols = np.ascontiguousarray(
        mask.reshape(B, NKC, 128).transpose(2, 0, 1).reshape(128, B * NKC)
    )
    in_maps = []
    for c in range(NCORES):
        sl = slice(c * DPC, (c + 1) * DPC)
        def warr(W):
            return np.ascontiguousarray(
                W[:, sl].astype(np.float16).reshape(NCI, 128, DPC)
                .transpose(1, 0, 2)
            )
        in_maps.append({
            "xtt": xtt,
            "wq": warr(Wq),
            "wk": warr(Wk),
            "wv": warr(Wv),
            "bq": np.ascontiguousarray(bq[sl, None]),
            "bk": np.ascontiguousarray(bk[sl, None]),
            "bvb": np.ascontiguousarray(
                np.broadcast_to(bv[sl][None, :], (128, DPC))
            ),
            "msk": mcols,
        })

    res = run_bass_kernel_spmd(
        nc, in_maps, core_ids=list(range(NCORES)), trace=trace
    )
    last_exec_time_ns = res.exec_time_ns
    last_results = res

    # assemble: per-core out [128, T] -> [B, S, 128]; concat over cores
    parts = [
        res.results[c]["out"].astype(np.float32).reshape(DPC, B, S)
        .transpose(1, 2, 0)
        for c in range(NCORES)
    ]
    return np.ascontiguousarray(np.concatenate(parts, axis=2))
